# revision 1
# baseline (speedup 1.0000x reference)
"""nn_Decoder: LSTM decoder with attention on 8 TRN2 NeuronCores.

Sharding: tensor-parallel over the 4H LSTM gate dim (each core owns a 128-wide
H-chunk = 512 gate rows, gate order (i,f,o,g)); one AllGather of the h.T
chunks per tick (layer-pipelined: tick t gathers {h0(t), h1(t-1)} together).
Layer-0 input-side gates (emb @ W_ih0.T + b) are bulk matmuls interleaved
into the AllGather latency windows. Attention + output projection run after
the recurrence, batched over all T, sharded over batch (8 per core).
Matmul operands use float32r (full-rate PE streaming, fp32 storage).
"""
import sys
sys.path.insert(0, "/opt/trn_rl_repo")

import numpy as np
import concourse.bass as bass
import concourse.bacc as bacc
import concourse.tile as tile
import concourse.mybir as mybir
from concourse.bass_utils import run_bass_kernel_spmd

F32 = mybir.dt.float32
F32R = mybir.dt.float32r
AF = mybir.ActivationFunctionType
ALU = mybir.AluOpType
X = mybir.AxisListType.X

T, B, S = 64, 64, 64
H = 1024
E = 1024
L = 2
V = 32000
NC = 8
BL = B // NC
GC = 4 * H // NC
KT = H // 128
CORE_IDS = list(range(NC))


def _host_prep_v1(inputs, t_steps=T):
    """inputs: dict from setup_inputs(). Returns in_maps for 8 cores."""
    idx = np.asarray(inputs["rnn_inputs"]).astype(np.int64)[:t_steps]
    emb = np.asarray(inputs["emb"], np.float32)
    embedded = emb[idx]                                 # [t, B, E]
    embT = np.ascontiguousarray(embedded.reshape(t_steps * B, E).T)
    ctx = np.asarray(inputs["context"], np.float32)     # [S, B, H]
    W_ih = np.asarray(inputs["W_ih"], np.float32)
    W_hh = np.asarray(inputs["W_hh"], np.float32)
    b_ih = np.asarray(inputs["b_ih"], np.float32)
    b_hh = np.asarray(inputs["b_hh"], np.float32)
    h0 = np.asarray(inputs["h0"], np.float32)
    c0 = np.asarray(inputs["c0"], np.float32)
    W_in = np.asarray(inputs["W_in"], np.float32)
    W_out = np.asarray(inputs["W_out"], np.float32)

    i64 = np.eye(64, dtype=np.float32)
    i128 = np.eye(128, dtype=np.float32)
    winT = np.ascontiguousarray(W_in.T)
    woutT = np.ascontiguousarray(W_out.T)

    in_maps = []
    for r in range(NC):
        blk = np.arange(128 * r, 128 * (r + 1))
        rows = np.concatenate([0 * H + blk, 1 * H + blk, 3 * H + blk, 2 * H + blk])
        bsl = slice(BL * r, BL * (r + 1))
        m = {
            "embT": embT,
            "wih0": np.ascontiguousarray(W_ih[0][rows].T),
            "whh0": np.ascontiguousarray(W_hh[0][rows].T),
            "wih1": np.ascontiguousarray(W_ih[1][rows].T),
            "whh1": np.ascontiguousarray(W_hh[1][rows].T),
            "bias0": np.broadcast_to((b_ih[0] + b_hh[0])[rows], (128, GC)).copy(),
            "bias1": np.broadcast_to((b_ih[1] + b_hh[1])[rows], (B, GC)).copy(),
            "h0t0": np.ascontiguousarray(h0[0].T),
            "h1t0": np.ascontiguousarray(h0[1].T),
            "c0c0": np.ascontiguousarray(c0[0][:, 128 * r:128 * (r + 1)]),
            "c1c0": np.ascontiguousarray(c0[1][:, 128 * r:128 * (r + 1)]),
            "ctxT": np.ascontiguousarray(ctx[:, bsl, :].transpose(1, 2, 0)),
            "ctxS": np.ascontiguousarray(ctx[:, bsl, :].transpose(1, 0, 2)),
            "winT": winT,
            "woutT": woutT,
            "i64": i64,
            "i128": i128,
        }
        in_maps.append(m)
    return in_maps



def host_prep(inputs, t_steps=T):
    in_maps = _host_prep_v1(inputs, t_steps)
    h0 = np.asarray(inputs["h0"], np.float32)
    for r in range(NC):
        in_maps[r]["h1ic"] = np.ascontiguousarray(
            h0[1].T[128 * r:128 * (r + 1), :])
    return in_maps


def assemble(results, t_steps=T):
    """results: per-core dicts -> (outputs, (hT, cT), attn_last)."""
    outputs = np.zeros((t_steps, B, H), np.float32)
    cT = np.zeros((L, B, H), np.float32)
    attn_last = np.zeros((B, S), np.float32)
    for r in range(NC):
        outT = results[r]["outT"]        # [H, BL*t]
        v = outT.reshape(H, BL, t_steps).transpose(2, 1, 0)  # [t, BL, H]
        outputs[:, BL * r:BL * (r + 1), :] = v
        cT[:, :, 128 * r:128 * (r + 1)] = results[r]["cT_c"]
        attn_last[BL * r:BL * (r + 1), :] = results[r]["attn"]
    hT = results[0]["hT_f"].transpose(0, 2, 1)  # [L, B, H]
    return outputs, (np.ascontiguousarray(hT), cT), attn_last


def build_nc(t_steps=T, reps=1, do_ag=True):
    nc = bacc.Bacc("TRN2", target_bir_lowering=False, debug=False,
                   enable_asserts=True, num_devices=NC, enable_partition_id=True)
    MT = t_steps * B // 128
    TS = t_steps
    BT = BL * TS

    embT = nc.dram_tensor("embT", [E, t_steps * B], F32R, kind="ExternalInput")
    wih0 = nc.dram_tensor("wih0", [H, GC], F32R, kind="ExternalInput")
    whh0 = nc.dram_tensor("whh0", [H, GC], F32R, kind="ExternalInput")
    wih1 = nc.dram_tensor("wih1", [H, GC], F32R, kind="ExternalInput")
    whh1 = nc.dram_tensor("whh1", [H, GC], F32R, kind="ExternalInput")
    bias0 = nc.dram_tensor("bias0", [128, GC], F32R, kind="ExternalInput")
    bias1 = nc.dram_tensor("bias1", [B, GC], F32R, kind="ExternalInput")
    h0t0 = nc.dram_tensor("h0t0", [H, B], F32R, kind="ExternalInput")
    h1t0 = nc.dram_tensor("h1t0", [H, B], F32R, kind="ExternalInput")
    h1ic = nc.dram_tensor("h1ic", [128, B], F32R, kind="ExternalInput")
    c0c0 = nc.dram_tensor("c0c0", [B, 128], F32, kind="ExternalInput")
    c1c0 = nc.dram_tensor("c1c0", [B, 128], F32, kind="ExternalInput")
    ctxT = nc.dram_tensor("ctxT", [BL, H, S], F32, kind="ExternalInput")
    ctxS = nc.dram_tensor("ctxS", [BL, S, H], F32, kind="ExternalInput")
    winT = nc.dram_tensor("winT", [H, H], F32R, kind="ExternalInput")
    woutT = nc.dram_tensor("woutT", [2 * H, H], F32R, kind="ExternalInput")
    i64 = nc.dram_tensor("i64", [64, 64], F32R, kind="ExternalInput")
    i128 = nc.dram_tensor("i128", [128, 128], F32R, kind="ExternalInput")

    outT = nc.dram_tensor("outT", [H, BT], F32, kind="ExternalOutput")
    hT_f = nc.dram_tensor("hT_f", [L, H, B], F32, kind="ExternalOutput")
    cT_c = nc.dram_tensor("cT_c", [L, B, 128], F32, kind="ExternalOutput")
    attn = nc.dram_tensor("attn", [BL, S], F32, kind="ExternalOutput")

    agi = nc.dram_tensor("agi", [t_steps + 1, 256, B], F32R)
    ago = nc.dram_tensor("ago", [t_steps + 1, 2 * H, B], F32R, addr_space="Shared")

    with tile.TileContext(nc) as tc:
      for _rep in range(reps):
        with (
            tc.tile_pool(name=f"const{_rep}", bufs=1) as cpool,
            tc.tile_pool(name=f"state{_rep}", bufs=1) as spool,
        ):
            i64_sb = cpool.tile([64, 64], F32R)
            nc.sync.dma_start(i64_sb[:].bitcast(F32R), i64[:, :])
            i128_sb = cpool.tile([128, 128], F32R)
            nc.sync.dma_start(i128_sb[:].bitcast(F32R), i128[:, :])
            b0_sb = cpool.tile([128, GC], F32R)
            nc.sync.dma_start(b0_sb[:].bitcast(F32R), bias0[:, :])
            b1_sb = cpool.tile([B, GC], F32R)
            nc.sync.dma_start(b1_sb[:].bitcast(F32R), bias1[:, :])

            h_sb = [spool.tile([128, KT * 64], F32R, tag=f"h{l}", name=f"h{l}")
                    for l in range(L)]
            for l, src in enumerate((h0t0, h1t0)):
                nc.sync.dma_start(
                    h_sb[l][:].bitcast(F32R).rearrange("p (k b) -> p k b", k=KT),
                    src.ap().rearrange("(k p) b -> p k b", p=128))
            c_sb = [spool.tile([B, 128], F32, tag=f"c{l}", name=f"c{l}")
                    for l in range(L)]
            for l, src in enumerate((c0c0, c1c0)):
                nc.sync.dma_start(c_sb[l][:], src[:, :])
            qT = spool.tile([128, KT * BT], F32R, tag="qT")

            pid8 = nc.vector.partition_id() * BL

            with (
                tc.tile_pool(name=f"wpool{_rep}", bufs=1) as wpool,
                tc.tile_pool(name=f"pb_in{_rep}", bufs=3) as pbi,
                tc.tile_pool(name=f"pc_cell{_rep}", bufs=2) as pcell,
                tc.tile_pool(name=f"pc_ps{_rep}", bufs=2, space="PSUM") as pps,
                tc.tile_pool(name=f"pc_pst{_rep}", bufs=2, space="PSUM") as ppst,
                tc.tile_pool(name=f"pb_ps{_rep}", bufs=2, space="PSUM") as pbp,
            ):
                def load_w(t_dram, name):
                    w = wpool.tile([128, KT * GC], F32R, tag=name, name=name + "_sb")
                    nc.sync.dma_start(
                        w[:].bitcast(F32R).rearrange("p (k g) -> p k g", k=KT),
                        t_dram.ap().rearrange("(k p) g -> p k g", p=128))
                    return w

                wih0_sb = load_w(wih0, "wih0")
                whh0_sb = load_w(whh0, "whh0")
                wih1_sb = load_w(wih1, "wih1")
                whh1_sb = load_w(whh1, "whh1")
                g0_sb = [wpool.tile([128, GC], F32R, tag=f"g0_{m}", name=f"g0_{m}")
                         for m in range(MT)]
                h1ics = wpool.tile([128, B], F32R, tag="h1ics", name="h1ics")
                nc.sync.dma_start(h1ics[:], h1ic[:, :])
                def g0_compute(m):
                    ek = pbi.tile([128, KT * 128], F32R, tag="ek", name="ek")
                    nc.sync.dma_start(
                        ek[:].bitcast(F32R).rearrange("p (k c) -> p k c", k=KT),
                        embT.ap()[:, m * 128:(m + 1) * 128]
                            .rearrange("(k p) c -> p k c", p=128))
                    psb = pbp.tile([128, GC], F32, tag="psb", name="psb", bufs=1)
                    nc.tensor.matmul(psb[:], i128_sb[:].bitcast(F32R),
                                     b0_sb[:].bitcast(F32R),
                                     start=True, stop=False)
                    for k in range(KT):
                        nc.tensor.matmul(
                            psb[:], ek[:, k * 128:(k + 1) * 128].bitcast(F32R),
                            wih0_sb[:, k * GC:(k + 1) * GC].bitcast(F32R),
                            start=False, stop=(k == KT - 1))
                    nc.scalar.copy(g0_sb[m][:].bitcast(F32R), psb[:])

                def cell(l, ps, t_idx):
                    sig = pcell.tile([B, 384], F32, tag=f"sig{l}", name=f"sig{l}")
                    nc.scalar.activation(sig[:], ps[:, 0:384], AF.Sigmoid)
                    tg = pcell.tile([B, 128], F32, tag=f"tg{l}", name=f"tg{l}")
                    nc.scalar.activation(tg[:], ps[:, 384:512], AF.Tanh)
                    t1 = pcell.tile([B, 128], F32, tag=f"t1{l}", name=f"t1{l}")
                    nc.vector.tensor_mul(t1[:], sig[:, 128:256], c_sb[l][:])
                    t2 = pcell.tile([B, 128], F32, tag=f"t2{l}", name=f"t2{l}")
                    nc.vector.tensor_mul(t2[:], sig[:, 0:128], tg[:])
                    nc.vector.tensor_add(c_sb[l][:], t1[:], t2[:])
                    tc1 = pcell.tile([B, 128], F32, tag=f"tc1{l}", name=f"tc1{l}")
                    nc.scalar.activation(tc1[:], c_sb[l][:], AF.Tanh)
                    hn = pcell.tile([B, 128], F32, tag=f"hn{l}", name=f"hn{l}")
                    nc.vector.tensor_mul(hn[:], sig[:, 256:384], tc1[:])
                    return hn

                g0_compute(0)
                g0_compute(1)

                for t in range(t_steps + 1):
                    # ---- layer-0 gate matmuls for h0(t) ----
                    hn0 = None
                    if t < t_steps:
                        ps0 = pps.tile([B, GC], F32, tag="ps0", name="ps0")
                        m, half = divmod(t, 2)
                        nc.tensor.matmul(
                            ps0[:], i128_sb[:, half * B:(half + 1) * B].bitcast(F32R),
                            g0_sb[m][:].bitcast(F32R),
                            start=True, stop=False)
                        for k in range(KT):
                            nc.tensor.matmul(
                                ps0[:], h_sb[0][:, k * 64:(k + 1) * 64].bitcast(F32R),
                                whh0_sb[:, k * GC:(k + 1) * GC].bitcast(F32R),
                                start=False, stop=(k == KT - 1))
                        hn0 = cell(0, ps0, t)
                    # ---- layer-1 gate matmuls for h1(t-1) ----
                    hn1 = None
                    if t >= 1:
                        ps1 = pps.tile([B, GC], F32, tag="ps1", name="ps1")
                        nc.tensor.matmul(ps1[:], i64_sb[:].bitcast(F32R),
                                         b1_sb[:].bitcast(F32R),
                                         start=True, stop=False)
                        for k in range(KT):
                            nc.tensor.matmul(
                                ps1[:], h_sb[1][:, k * 64:(k + 1) * 64].bitcast(F32R),
                                whh1_sb[:, k * GC:(k + 1) * GC].bitcast(F32R),
                                start=False, stop=False)
                        for k in range(KT):
                            nc.tensor.matmul(
                                ps1[:], h_sb[0][:, k * 64:(k + 1) * 64].bitcast(F32R),
                                wih1_sb[:, k * GC:(k + 1) * GC].bitcast(F32R),
                                start=False, stop=(k == KT - 1))
                        hn1 = cell(1, ps1, t - 1)
                    # ---- transposes + AG input ----
                    if hn0 is not None:
                        pst0 = ppst.tile([128, 64], F32, tag="pst0", name="pst0", bufs=1)
                        nc.tensor.transpose(pst0[:], hn0[:], i64_sb[:].bitcast(F32))
                        hch0 = pcell.tile([128, 64], F32R, tag="hch0", name="hch0")
                        nc.vector.tensor_copy(hch0[:], pst0[:])
                        nc.sync.dma_start(agi.ap()[t, 0:128, :], hch0[:])
                    else:
                        nc.sync.dma_start(agi.ap()[t, 0:128, :], h1ics[:])
                    if hn1 is not None:
                        pst1 = ppst.tile([128, 64], F32, tag="pst1", name="pst1", bufs=1)
                        nc.tensor.transpose(pst1[:], hn1[:], i64_sb[:].bitcast(F32))
                        hch1 = pcell.tile([128, 64], F32R, tag="hch1", name="hch1")
                        nc.vector.tensor_copy(hch1[:], pst1[:])
                        nc.sync.dma_start(agi.ap()[t, 128:256, :], hch1[:])
                    else:
                        nc.sync.dma_start(agi.ap()[t, 128:256, :], h1ics[:])
                    # ---- G0 precompute filling the AG window ----
                    if 2 <= t + 2 < MT:
                        g0_compute(t + 2)
                    # ---- collective ----
                    if do_ag:
                        nc.gpsimd.collective_compute(
                            "AllGather", ALU.bypass, replica_groups=[CORE_IDS],
                            ins=[agi.ap()[t]], outs=[ago.ap()[t]])
                    else:
                        nc.sync.dma_start(ago.ap()[t][0:256, :], agi.ap()[t])
                    # ---- state update from gathered ----
                    v = ago.ap()[t].rearrange("(k two p) b -> two p k b", p=128, two=2)
                    if t < t_steps:
                        nc.sync.dma_start(
                            h_sb[0][:].bitcast(F32R).rearrange("p (k b) -> p k b", k=KT), v[0])
                    if t >= 1:
                        nc.sync.dma_start(
                            h_sb[1][:].bitcast(F32R).rearrange("p (k b) -> p k b", k=KT), v[1])
                        nc.vector.tensor_copy(
                            qT[:].bitcast(F32R).rearrange("p (k b t2) -> p k b t2", k=KT, b=BL)
                                [:, :, :, t - 1],
                            h_sb[1][:].rearrange("p (k b) -> p k b", k=KT)
                                [:, :, bass.ds(pid8, BL)])

            for l in range(L):
                nc.sync.dma_start(
                    hT_f.ap()[l].rearrange("(k p) b -> p k b", p=128),
                    h_sb[l][:].bitcast(F32).rearrange("p (k b) -> p k b", k=KT))
                nc.sync.dma_start(cT_c.ap()[l], c_sb[l][:])

            # ================= Phase 3 (unchanged from v1) =================
            with (
                tc.tile_pool(name=f"p3w{_rep}", bufs=2) as p3w,
                tc.tile_pool(name=f"p3t{_rep}", bufs=2) as p3t,
                tc.tile_pool(name=f"p3ctx{_rep}", bufs=2) as p3c,
                tc.tile_pool(name=f"p3ps{_rep}", bufs=2, space="PSUM") as p3p,
                tc.tile_pool(name=f"p3res{_rep}", bufs=1) as p3r,
            ):
                uT = p3r.tile([128, KT * BT], F32, tag="uT")
                for m in range(KT):
                    wmt = p3w.tile([128, KT * 128], F32R, tag="wmt", name="wmt")
                    nc.sync.dma_start(
                        wmt[:].bitcast(F32R).rearrange("p (k c) -> p k c", k=KT),
                        winT.ap()[:, m * 128:(m + 1) * 128]
                            .rearrange("(k p) c -> p k c", p=128))
                    psu = p3p.tile([128, BT], F32, tag="psu", name="psu")
                    for k in range(KT):
                        nc.tensor.matmul(
                            psu[:], wmt[:, k * 128:(k + 1) * 128].bitcast(F32R),
                            qT[:, k * BT:(k + 1) * BT].bitcast(F32R),
                            start=(k == 0), stop=(k == KT - 1))
                    nc.scalar.copy(uT[:, m * BT:(m + 1) * BT], psu[:])

                ctxT_sb = p3r.tile([128, BL * KT * 64], F32, tag="ctxT")
                nc.sync.dma_start(
                    ctxT_sb[:].rearrange("p (b k s) -> p b k s", b=BL, k=KT),
                    ctxT.ap().rearrange("b (k p) s -> p b k s", p=128))

                wT = p3r.tile([128, KT * BT], F32R, tag="wT")
                for b in range(BL):
                    pss = p3p.tile([TS, S], F32, tag="pss", bufs=1, name="pss")
                    for k in range(KT):
                        nc.tensor.matmul(
                            pss[:], uT[:, k * BT + TS * b:k * BT + TS * (b + 1)],
                            ctxT_sb[:, (b * KT + k) * 64:(b * KT + k + 1) * 64],
                            start=(k == 0), stop=(k == KT - 1))
                    nmx = p3t.tile([TS, 1], F32, tag="nmx", name="nmx")
                    nc.vector.tensor_reduce(nmx[:], pss[:], X, ALU.max, negate=True)
                    ex = p3t.tile([TS, S], F32, tag="ex", name="ex")
                    sm = p3t.tile([TS, 1], F32, tag="sm", name="sm")
                    nc.scalar.activation(ex[:], pss[:], AF.Exp, bias=nmx[:],
                                         accum_out=sm[:])
                    rs = p3t.tile([TS, 1], F32, tag="rs", name="rs")
                    nc.vector.reciprocal(rs[:], sm[:])
                    ab = p3t.tile([TS, S], F32, tag="ab", name="ab")
                    nc.vector.tensor_scalar_mul(ab[:], ex[:], rs[:])
                    nc.sync.dma_start(attn.ap()[b:b + 1, :], ab[TS - 1:TS, :])
                    psa = p3p.tile([S, TS], F32, tag="psa", bufs=1, name="psa")
                    nc.tensor.transpose(psa[:], ab[:], i64_sb[0:TS, 0:TS].bitcast(F32))
                    abT = p3t.tile([S, TS], F32, tag="abT", name="abT")
                    nc.scalar.copy(abT[:], psa[:])
                    ctxs = p3c.tile([S, H], F32, tag="ctxs", name="ctxs")
                    nc.sync.dma_start(ctxs[:], ctxS.ap()[b])
                    psw = p3p.tile([128, KT * TS], F32, tag="psw", bufs=1, name="psw")
                    for m in range(KT):
                        nc.tensor.matmul(
                            psw[:, m * TS:(m + 1) * TS],
                            ctxs[:, m * 128:(m + 1) * 128], abT[:],
                            start=True, stop=True)
                    nc.vector.tensor_copy(
                        wT[:].bitcast(F32R).rearrange("p (k bt) -> p k bt", k=KT)
                            [:, :, b * TS:(b + 1) * TS],
                        psw[:].rearrange("p (k t) -> p k t", k=KT))

                for m in range(KT):
                    wot = p3w.tile([128, 2 * KT * 128], F32R, tag="wot", name="wot")
                    nc.sync.dma_start(
                        wot[:].bitcast(F32R).rearrange("p (k c) -> p k c", k=2 * KT),
                        woutT.ap()[:, m * 128:(m + 1) * 128]
                            .rearrange("(k p) c -> p k c", p=128))
                    pso = p3p.tile([128, BT], F32, tag="pso", name="pso")
                    for k in range(KT):
                        nc.tensor.matmul(
                            pso[:], wot[:, k * 128:(k + 1) * 128].bitcast(F32R),
                            wT[:, k * BT:(k + 1) * BT].bitcast(F32R),
                            start=(k == 0), stop=False)
                    for k in range(KT):
                        nc.tensor.matmul(
                            pso[:], wot[:, (KT + k) * 128:(KT + k + 1) * 128].bitcast(F32R),
                            qT[:, k * BT:(k + 1) * BT].bitcast(F32R),
                            start=False, stop=(k == KT - 1))
                    oo = p3t.tile([128, BT], F32, tag="oo", name="oo")
                    nc.scalar.activation(oo[:], pso[:], AF.Tanh)
                    nc.sync.dma_start(outT.ap()[m * 128:(m + 1) * 128, :], oo[:])

    nc.compile()
    return nc


_NC_CACHE = {}


def _get_nc(reps=1):
    if reps not in _NC_CACHE:
        _NC_CACHE[reps] = build_nc(reps=reps)
    return _NC_CACHE[reps]


def kernel(**inputs):
    nc = _get_nc()
    in_maps = host_prep(inputs)
    res = run_bass_kernel_spmd(nc, in_maps, CORE_IDS)
    return assemble(res.results)
